# revision 46
# baseline (speedup 1.0000x reference)
"""Trainium2 Bass kernel for nn_BGATTNET_Loss (CE + pairwise cosine-sim regularizer).

Math
----
loss = CE(outputs, labels) + sum_b [ COE/n_pairs * sum_{i<j} cos(H[b,i], H[b,j]) ]

The O(N^2 D) pairwise term collapses to O(N D):
    sum_{i<j} cos_ij = 0.5 * ( || sum_n Hn_n ||^2  -  sum_n ||Hn_n||^2 )
with Hn_n = H_n / max(||H_n||, EPS).

Sharding: data-parallel over the bag dim B=8, one bag per NeuronCore.
Each core computes   partial_b = CE_b/8 + 0.5*COE/n_pairs * (ssq_b - N)
and the host sums the 8 scalars (rowssq is analytically N: unit-norm rows).

Per-core dataflow (bag H_b is [2048, 512] f32, streamed once = 4 MB,
which is the per-core HBM roofline ~12us at ~341 GB/s):
  - HWDGE DMA in tapered chunks (4,4,4,2,1,1 row-tiles of [128, 512]) so
    the stream saturates bandwidth while the post-last-byte tail is short
  - per-row sum-of-squares: ACT Square+accum_out on 6 tiles, DVE fused
    square+reduce (TensorScalarPtr w/ accum) on 10 (engine balance)
  - rnorm = 1/max(sqrt(sumsq), EPS) via int-magic + 1 Newton step on DVE
    only (no ACT table thrash; rel err <2e-3, irrelevant at reg's weight)
  - weighted column sum s = sum_n rnorm_n * H_n on the PE: per-tile
    matmul with the rnorm column as stationary operand, float32r (raw
    fp32, 1 cycle/row) accumulated in PSUM [1, 512]
  - ssq = CREG*||s||^2 read straight from PSUM by one ACT Square+accum
  - CE on-device: exp+accum -> ln on ACT (one table set pair, loaded in
    the first DMA's shadow), label select and combine on DVE
"""

from contextlib import ExitStack

import numpy as np

import concourse.bass as bass
import concourse.tile as tile
from concourse import bacc, mybir
from concourse._compat import axon_active
from concourse.bass_utils import run_bass_kernel_spmd

P = 128
B = 8
N = 2048
D = 512
NT = N // P  # 16 row tiles
G = 4  # row tiles per DMA group (1 MB)
NG = NT // G

COE = 0.01
N_PAIRS = N * (N - 1) / 2.0
CREG = float(0.5 * COE / N_PAIRS)
EPS = 1e-12

F32 = mybir.dt.float32
BF16 = mybir.dt.bfloat16
F32R = mybir.dt.float32r
I32 = mybir.dt.int32
AF = mybir.ActivationFunctionType
ALU = mybir.AluOpType

# Tiles whose sum-of-squares runs on ACT (Square+accum); the rest go to
# DVE (fused square+reduce). Balanced from the cost model (ACT ~799ns vs
# DVE ~594ns per tile); the final tile stays on DVE to keep the tail short.
ACT_SQ_TILES = frozenset({0, 3, 6, 9, 14, 15})

# int bit-trick seed for Newton rsqrt on DVE, pre-adjusted for a halved
# input: y0 = bits(MAGIC2 - (bits(x/2) >> 1)) approximates 1/sqrt(x)
RSQRT_MAGIC2 = 0x5F3759DF - 0x00400000


def _build_bass():
    nc = bacc.Bacc(
        "TRN2",
        target_bir_lowering=False,
        debug=not axon_active(),
        enable_asserts=False,
        num_devices=B,
    )

    # h is declared float32r (same bytes as f32): the PE's raw-fp32 matmul
    # mode needs f32r-typed producers end-to-end per the BIR verifier
    h = nc.dram_tensor("h", [N, D], F32R, kind="ExternalInput")
    xl_in = nc.dram_tensor("xl_in", [1, 3], F32, kind="ExternalInput")
    out = nc.dram_tensor("partial", [1, 1], F32, kind="ExternalOutput")

    hv = h[:, :].rearrange("(t p) d -> p t d", p=P)  # [128, 16, 512]

    with tile.TileContext(nc) as tc, ExitStack() as ctx:
        hpool = ctx.enter_context(tc.tile_pool(name="hbuf", bufs=6))
        scr_act = ctx.enter_context(tc.tile_pool(name="scr_act", bufs=2))
        scr_dve = ctx.enter_context(tc.tile_pool(name="scr_dve", bufs=2))
        grp = ctx.enter_context(tc.tile_pool(name="grp", bufs=2))
        stats = ctx.enter_context(tc.tile_pool(name="stats", bufs=1))
        small = ctx.enter_context(tc.tile_pool(name="small", bufs=1))
        psum = ctx.enter_context(tc.tile_pool(name="psum", bufs=1, space="PSUM"))

        sumsq = stats.tile([P, NT], F32)  # per-row ||H_n||^2
        rnorm = stats.tile([P, NT], F32R)  # per-row 1/max(||H_n||, EPS)
        magic = stats.tile([P, NT], I32)
        nc.vector.memset(magic, RSQRT_MAGIC2)

        s_acc = psum.tile([1, D], F32)  # sum_n rnorm_n * H_n

        # ---- CE for this core's bag (tiny; high priority so both ACT
        # table loads land in the early DMA shadow). lse computed without
        # max-shift (|outputs| ~ N(0,1), exp is safe in f32) so Exp and Ln
        # are adjacent ACT ops with no DVE round-trip between them. ----
        with tc.high_priority():
            x_sb = small.tile([1, 3], F32)
            nc.sync.dma_start(out=x_sb, in_=xl_in[:, :])

            e = small.tile([1, 2], F32)
            se = small.tile([1, 1], F32)
            nc.scalar.activation(e, x_sb[:, 0:2], AF.Exp, accum_out=se)
            lse = small.tile([1, 1], F32)
            lse_inst = nc.scalar.activation(lse, se, AF.Ln)
            dx = small.tile([1, 1], F32)
            nc.vector.tensor_tensor(dx, x_sb[:, 1:2], x_sb[:, 0:1], ALU.subtract)
            xl = small.tile([1, 1], F32)
            nc.vector.scalar_tensor_tensor(
                xl, in0=dx, scalar=x_sb[:, 2:3], in1=x_sb[:, 0:1],
                op0=ALU.mult, op1=ALU.add,
            )
            ce = small.tile([1, 1], F32)
            nc.vector.tensor_tensor(ce, lse, xl, ALU.subtract)
            ce8 = small.tile([1, 1], F32)
            nc.vector.tensor_scalar_mul(ce8, ce, 1.0 / B)
            # bias for the final ACT combine: ce/8 - CREG*N (rowssq = N)
            bias_pre = small.tile([1, 1], F32)
            nc.vector.tensor_scalar(
                bias_pre, in0=ce8, scalar1=float(N * CREG), scalar2=None,
                op0=ALU.subtract,
            )

        # ---- stream H: sumsq -> rnorm -> PE weighted column-sum ----
        def newton_rsqrt(ph):
            """rnorm[:, ph] = 1/sqrt(max(sumsq[:, ph], EPS^2)) on DVE only:
            quake int-magic seed + 1 Newton iteration (rel err <2e-3, well
            under the bf16 rounding the matmul already applies)."""
            w = ph.stop - ph.start
            xh = grp.tile([P, w], F32)
            nc.vector.tensor_scalar(
                xh, in0=sumsq[:, ph], scalar1=EPS * EPS, scalar2=0.5,
                op0=ALU.max, op1=ALU.mult,
            )
            yi = grp.tile([P, w], I32)
            nc.vector.tensor_scalar(
                yi, in0=xh[:, :].bitcast(I32), scalar1=1, scalar2=None,
                op0=ALU.arith_shift_right,
            )
            nc.vector.tensor_tensor(yi, magic[:, ph], yi, ALU.subtract)
            y = yi[:, :].bitcast(F32)
            a = grp.tile([P, w], F32)
            nc.vector.tensor_mul(a, y, y)
            nc.vector.tensor_mul(a, a, xh)
            nc.vector.tensor_scalar(
                a, in0=a, scalar1=-1.0, scalar2=1.5, op0=ALU.mult, op1=ALU.add
            )
            nc.vector.tensor_mul(rnorm[:, ph], y, a)  # f32 -> f32r out

        # tapered DMA chunks (big for bandwidth, small at the end) with
        # compute phases decoupled: each phase = rsqrt chain + matmuls over
        # tiles whose chunks have landed. The last two phases are narrow so
        # the after-last-byte tail is short.
        chunks = [(0, 4), (4, 8), (8, 12), (12, 14), (14, 15), (15, NT)]
        phases = [(0, 4), (4, 8), (8, 12), (12, 14), (14, NT)]
        hts = {}
        pidx = 0
        for lo, hi in chunks:
            ht = hpool.tile([P, hi - lo, D], F32R, tag="hbuf")
            nc.sync.dma_start(out=ht, in_=hv[:, lo:hi, :])

            for j in range(hi - lo):
                t = lo + j
                hts[t] = (ht, j)
                if t in ACT_SQ_TILES:
                    # ACT path: Square with free-dim accumulate
                    sa = scr_act.tile([P, D], F32)
                    sq_inst = nc.scalar.activation(
                        sa, ht[:, j, :].bitcast(F32), AF.Square,
                        accum_out=sumsq[:, t : t + 1],
                    )
                    if t == 0:
                        # order-only edge: CE's Ln (and its table load) must
                        # precede the first square so the natural_log table
                        # load lands in the early DMA shadow, not mid-stream
                        tile.add_dep_helper(
                            sq_inst.ins, lse_inst.ins, sync=False,
                            reason="ACT table load before square stream",
                        )
                else:
                    # DVE path: fused square+reduce (TensorScalarPtr w/ accum)
                    sv = scr_dve.tile([P, D], F32)
                    nc.vector.scalar_tensor_tensor(
                        sv, in0=ht[:, j, :].bitcast(F32), scalar=1.0,
                        in1=ht[:, j, :].bitcast(F32),
                        op0=ALU.mult, op1=ALU.mult,
                        accum_out=sumsq[:, t : t + 1],
                    )

            while pidx < len(phases) and phases[pidx][1] <= hi:
                plo, phi = phases[pidx]
                pidx += 1
                newton_rsqrt(slice(plo, phi))
                for t in range(plo, phi):
                    tht, j = hts[t]
                    nc.tensor.matmul(
                        s_acc[:, :],
                        lhsT=rnorm[:, t : t + 1],
                        rhs=tht[:, j, :],
                        start=(t == 0),
                        stop=(t == NT - 1),
                    )

        # ---- finals, all on ACT so no cross-engine hop before the output:
        # ssq = CREG*||s||^2 straight from PSUM (Square of sqrt(CREG)*s with
        # accumulate), then partial = ssq + (ce/8 - CREG*N) via Identity
        sq_s = psum.tile([1, D], F32)
        ssq = small.tile([1, 1], F32)
        nc.scalar.activation(
            sq_s, s_acc, AF.Square, scale=float(np.sqrt(CREG)), accum_out=ssq
        )
        part = small.tile([1, 1], F32)
        nc.scalar.activation(part, ssq, AF.Identity, bias=bias_pre[:, :])
        nc.scalar.dma_start(out=out[:, :], in_=part)

    nc.compile()
    return nc


_NC_CACHE = None


def _get_nc():
    global _NC_CACHE
    if _NC_CACHE is None:
        _NC_CACHE = _build_bass()
    return _NC_CACHE


def _run(inputs, trace=False, **kwargs):
    outputs = np.asarray(inputs["outputs"], dtype=np.float32)
    labels = np.asarray(inputs["labels"])
    H = np.asarray(inputs["H"], dtype=np.float32)
    assert H.shape == (B, N, D), H.shape

    in_maps = []
    for b in range(B):
        in_maps.append(
            {
                "h": np.ascontiguousarray(H[b]),
                "xl_in": np.array(
                    [[outputs[b, 0], outputs[b, 1], float(labels[b])]],
                    dtype=np.float32,
                ),
            }
        )
    res = run_bass_kernel_spmd(
        _get_nc(), in_maps, core_ids=list(range(B)), trace=trace, **kwargs
    )
    partials = [float(r["partial"][0, 0]) for r in res.results]
    total = np.float32(sum(partials))
    return np.asarray(total, dtype=np.float32), res


def kernel(**inputs) -> np.ndarray:
    total, _ = _run(inputs, trace=False)
    return total
